# revision 44
# baseline (speedup 1.0000x reference)
"""Distributed Trainium2 kernel for GPT-2 style multi-head causal attention.

reference:
    qkv = x @ w_attn + b_attn            # [B,S,3*NX]
    q,k,v split; 16 heads, DH=64; causal softmax(q k^T / sqrt(DH)) v
    out = a @ w_proj + b_proj            # [B,S,NX]

Sharding over 8 NeuronCores: core c -> (batch b=c//2, head-group g=c%2).
Each core computes qkv for its batch and its 8 heads (Megatron column-parallel
c_attn), flash attention for those 8 heads fully in SBUF, a per-head-pair
2-core AllGather of the attention outputs (overlapped under later pairs'
compute), and a column-parallel c_proj (each core produces 512 of the 1024
output features for all 2048 tokens of its batch). Host concatenates.

v3-v7 changes vs the ~421us v2 baseline (measured ~330us median):
- GEMM1a emitted at (fc, 512-token) granularity: 8 contraction matmuls into
  one [128,512] psum + one epilogue. Halves the Ldweights count (512->256)
  and the epilogue instruction count; pieces stay ~0.85us so they still
  serve as pacer fillers.
- wqk shipped host-side in fc-major layout [128, fc, c, 128] so per-fc DMAs
  have 2KB contiguous runs (the old [128, c, 1024] layout hit the <512B
  read-modify-write 2x DMA penalty on every slice).
- intro DMAs split across both HWDGE queues (SP + Activation) at c-pair
  granularity: doubles the ~0.65us-per-DMA issue rate and lets hardware
  overlap the two queues' transfers. The preflash c-loop (von tt0/tt1 +
  q/k fc0) paces right behind the stream; von tt2/tt3 moved into pair-0's
  qt0 fillers. First exp fires ~5us earlier than v2.
- flash software pipeline deepened to lag 2 (PV trails scores by two
  chunks) and, for pairs 1-3, diagonal chunks processed early in each qt:
  the exp->mask->PV chain of masked chunks gets ~1 extra chunk of slack,
  removing the ~0.1-0.6us PE stalls per chunk seen in the v2 trace.
- pair-3 scheduling decoupled from the collective chain: pass1 tt12-15
  fills qt0 (no dependence on the just-issued pair-2 AllGather), {2,6}
  and {3,7} chunks fill qt1+ behind their respective AllGathers, and the
  pair-3 normalize uses a PE-matmul reciprocal-broadcast (not gpsimd) so
  the Pool-engine collectives can't block it.
- AllGather granularity is selectable (ag="ag3"|"m4"|"qt"); measured on
  hardware the 7-collective "ag3" split is best: 16 per-qt collectives
  pay ~3-5us fixed cost each (invisible to the cost-model sim, which
  models collectives as local DMAs), while 4 merged ones lose overlap.
"""

import sys

if "/opt/trn_rl_repo" not in sys.path:
    sys.path.insert(0, "/opt/trn_rl_repo")

import numpy as np
import ml_dtypes

import concourse.bass as bass
import concourse.mybir as mybir
import concourse.tile as tile
from concourse import bacc
from concourse.bass_utils import run_bass_kernel_spmd

BF16 = ml_dtypes.bfloat16

B, S, NX, H = 4, 2048, 1024, 16
DH = NX // H  # 64
N_CORES = 8
HPC = 8          # heads per core
FQK = HPC * DH   # 512 q (or k) features per core
GQ = S // 512    # 4 q-tiles of 512
TT16 = S // 128  # 16 token chunks of 128

f32 = mybir.dt.float32
bf16 = mybir.dt.bfloat16

_BUILD_CACHE: dict = {}


def build_nc(reps: int = 1, sim_single: bool = False, no_cc: bool = False,
             ag: str = "ag3", tail2: bool = False, bulk2: bool = True,
             r0: bool = True):
    # tail2=True parks the 4 tail stores on the scalar DMA queue; measured
    # slower: back-to-back bodies then queue the next intro's scalar
    # stream (wv/wqk/msk) behind the dependency-gated stores.
    """Build + compile the SPMD Bass graph (identical on all 8 cores).

    reps>1 replicates the whole body (for slope-based timing).
    sim_single builds a 1-core variant with collectives replaced by
    equivalent local DMAs, for TimelineSim cost-model profiling.
    ag: "qt" = per-q-tile AllGather for every head pair (finest overlap,
    16 collectives); "ag3" = whole-pair AllGather for pairs 0-2, per-qt
    only for pair 3 (7 collectives).
    """
    key = ("nc", reps, sim_single, no_cc, ag, tail2, bulk2, r0)
    if key in _BUILD_CACHE:
        return _BUILD_CACHE[key]

    ndev = 1 if sim_single else N_CORES
    nc = bacc.Bacc("TRN2", target_bir_lowering=False, debug=False, num_devices=ndev)
    local_cc = sim_single or no_cc

    # all pre-rearranged host-side so DMAs have >=512B contiguous runs
    xT = nc.dram_tensor("xT", [128, 8, S], bf16, kind="ExternalInput")
    wqk = nc.dram_tensor("wqk", [128, 8, 8, 128], bf16, kind="ExternalInput")
    wv = nc.dram_tensor("wv", [128, 8, FQK], bf16, kind="ExternalInput")
    bqk = nc.dram_tensor("bqk", [128, 8], f32, kind="ExternalInput")
    bvb = nc.dram_tensor("bvb", [128, FQK], f32, kind="ExternalInput")
    wpj = nc.dram_tensor("wpj", [128, 8, FQK], bf16, kind="ExternalInput")
    bpj = nc.dram_tensor("bpj", [128, FQK], f32, kind="ExternalInput")
    msk = nc.dram_tensor("msk", [128, 4, 512], bf16, kind="ExternalInput")
    out = nc.dram_tensor("out", [S, FQK], f32, kind="ExternalOutput")

    groups = [[0, 1], [2, 3], [4, 5], [6, 7]]

    with tile.TileContext(nc) as tc:
      for _rep in range(reps):
        # internal DRAM for the collectives
        if ag == "qt":
            ag_ins_q = [[nc.dram_tensor(f"agi{_rep}_{i}_{q}", [128, 512], bf16)
                         for q in range(GQ)] for i in range(4)]
            ag_outs_q = [[nc.dram_tensor(f"ago{_rep}_{i}_{q}", [256, 512], bf16)
                          for q in range(GQ)] for i in range(4)]
        elif ag == "m4":
            ag_in01 = nc.dram_tensor(f"ag_in01{_rep}", [128, 2 * S], bf16)
            ag_out01 = nc.dram_tensor(f"ag_out01{_rep}", [256, 2 * S], bf16)
            ag_in2 = nc.dram_tensor(f"ag_in2{_rep}", [128, S], bf16)
            ag_out2 = nc.dram_tensor(f"ag_out2{_rep}", [256, S], bf16)
            ag_in3h = [nc.dram_tensor(f"ag_in3h{_rep}_{q}", [128, 1024], bf16)
                       for q in range(2)]
            ag_out3h = [nc.dram_tensor(f"ag_out3h{_rep}_{q}", [256, 1024], bf16)
                        for q in range(2)]
        else:
            ag_ins = [nc.dram_tensor(f"ag_in{_rep}_{i}", [128, S], bf16)
                      for i in range(3)]
            ag_outs = [nc.dram_tensor(f"ag_out{_rep}_{i}", [256, S], bf16)
                       for i in range(3)]
            ag_ins_q = [None, None, None,
                        [nc.dram_tensor(f"agi{_rep}_3_{q}", [128, 512], bf16)
                         for q in range(GQ)]]
            ag_outs_q = [None, None, None,
                         [nc.dram_tensor(f"ago{_rep}_3_{q}", [256, 512], bf16)
                          for q in range(GQ)]]
        with (
            tc.tile_pool(name="persist", bufs=1) as pp,
            tc.tile_pool(name="ptmp", bufs=3) as ptmp,
            tc.tile_pool(name="ptmp2", bufs=2) as ptmp2,
            tc.tile_pool(name="oevict", bufs=2) as oev,
            tc.tile_pool(name="ps_s", bufs=2, space="PSUM") as ps_s,
            tc.tile_pool(name="ps_a", bufs=2, space="PSUM") as ps_a,
            tc.tile_pool(name="ps_g", bufs=2, space="PSUM") as ps_g,
        ):
            # ---- persistent SBUF tensors
            qkT = pp.tile([128, 8, S], bf16)
            von = pp.tile([128, TT16, HPC, 65], bf16)  # v natural + ones col
            aT = pp.tile([128, 4, S], bf16)        # per-head-pair attn out
            gath = pp.tile([128, 8, S], bf16)      # AllGathered aT
            og16 = pp.tile([128, TT16, FQK], bf16)  # c_proj pass1 partials
            msk_sb = pp.tile([128, 4, 512], bf16)
            bqk_sb = pp.tile([128, 8], f32)
            bvb_sb = pp.tile([128, FQK], f32)
            bpj_sb = pp.tile([128, FQK], f32)
            wpj_sb = pp.tile([128, 8, FQK], bf16)
            zb = pp.tile([128, 1], f32)
            ones64 = pp.tile([1, 64], f32)

            with tc.tile_pool(name="g1", bufs=1) as g1p:
                xT_sb = g1p.tile([128, 8, S], bf16)
                wv_sb = g1p.tile([128, 8, FQK], bf16)
                wqk_sb = g1p.tile([128, 8, 8, 128], bf16)

                # ---- input DMAs, ordered by first use. The preflash
                # c-loop below consumes {xT[c] q0, wv[c], wqk[fc0,c]} per
                # contraction chunk, so those stream first, interleaved.
                # intro split across both HWDGE queues (SP + Activation):
                # doubles the ~0.65us-per-DMA issue rate, and on hardware
                # the queues' transfers can overlap. The scalar queue must
                # drain before the first exp (~12us) — only preflash
                # tensors go there.
                nc.sync.dma_start(bqk_sb[:], bqk[:])
                nc.sync.dma_start(bvb_sb[:], bvb[:])
                for c in range(0, 8, 2):
                    nc.sync.dma_start(
                        xT_sb[:, c : c + 2, 0 : S // 4],
                        xT[:, c : c + 2, 0 : S // 4],
                    )
                    nc.scalar.dma_start(
                        wv_sb[:, c : c + 2, :], wv[:, c : c + 2, :]
                    )
                    nc.scalar.dma_start(
                        wqk_sb[:, 0, c : c + 2, :], wqk[:, 0, c : c + 2, :]
                    )

                def wqk_dma(fc):
                    nc.sync.dma_start(wqk_sb[:, fc, :, :], wqk[:, fc, :, :])

                # fc4 gates g1a(4,0) -> first scores; the sync queue has
                # fewer intro transfers queued than scalar
                wqk_dma(4)
                nc.scalar.dma_start(msk_sb[:], msk[:])

                def bulk_dma(q_, sl_):
                    # latest-needed bulk rides the scalar queue; these are
                    # dependency-free loads issued before the first exp,
                    # so they cannot block the ACT queue during flash
                    eng = nc.scalar if (bulk2 and q_ == "s") else nc.sync
                    eng.dma_start(*sl_)

                for fc in (1, 5):
                    wqk_dma(fc)
                sl1 = slice(S // 4, S // 2)
                nc.sync.dma_start(xT_sb[:, :, sl1], xT[:, :, sl1])
                for q in (2, 3):
                    sl = slice(q * (S // 4), (q + 1) * (S // 4))
                    bulk_dma("s", (xT_sb[:, :, sl], xT[:, :, sl]))
                for fc in (2, 3, 6, 7):
                    wqk_dma(fc)
                bulk_dma("s", (wpj_sb[:], wpj[:]))
                nc.sync.dma_start(bpj_sb[:], bpj[:])
                nc.vector.memset(zb[:], 0.0)
                nc.vector.memset(ones64[:], 1.0)
                nc.vector.memset(von[:, :, :, 64:65], 1.0)

                def g1a_piece(fc, tt):
                    # one (feature-chunk, 512-token) GEMM1a piece: 8
                    # contraction matmuls + bias epilogue, ~0.9us filler
                    cols = slice(tt * 512, (tt + 1) * 512)
                    ps = ps_g.tile([128, 512], f32, tag="g",
                                   name=f"g1a_{fc}_{tt}")
                    for c in range(8):
                        nc.tensor.matmul(
                            ps[:], wqk_sb[:, fc, c, :], xT_sb[:, c, cols],
                            start=(c == 0), stop=(c == 7),
                        )
                    nc.vector.tensor_scalar(
                        qkT[:, fc, cols], ps[:], bqk_sb[:, fc : fc + 1],
                        None, mybir.AluOpType.add,
                    )

                def g1b_epilogue(ps, tt):
                    nc.vector.tensor_tensor(
                        von[:, tt, :, 0:64],
                        ps[:].rearrange("p (h d) -> p h d", d=64),
                        bvb_sb[:].rearrange("p (h d) -> p h d", d=64),
                        mybir.AluOpType.add,
                    )

                def g1b_chunk(tt):
                    # v natural layout [t, h, d] (+ones col kept intact)
                    ps = ps_g.tile([128, 512], f32, tag="g")
                    for c in range(8):
                        nc.tensor.matmul(
                            ps[:],
                            xT_sb[:, c, tt * 128 : (tt + 1) * 128],
                            wv_sb[:, c, :],
                            start=(c == 0), stop=(c == 7),
                        )
                    g1b_epilogue(ps, tt)

                # ---- preflash, c-interleaved: for each contraction chunk
                # c, the von tt0/tt1 matmuls and the (fc0, tt0) q/k matmul
                # run right behind c's DMAs (three live psum accumulators:
                # 2 from ps_g, 1 from ps_a; flash tiles reuse those bufs
                # afterwards in order). von tt2/tt3 are pair-0 qt0 pacer
                # fillers — qt0's pv(kc=2,3) need them only mid-qt.
                gb_ps = [
                    ps_g.tile([128, 512], f32, tag="g", name="g1b_p0"),
                    ps_g.tile([128, 512], f32, tag="g", name="g1b_p1"),
                ]
                ga_ps = ps_a.tile([128, 512], f32, tag="aT", name="g1a00_p")
                for c in range(8):
                    for tt in range(2):
                        nc.tensor.matmul(
                            gb_ps[tt][:],
                            xT_sb[:, c, tt * 128 : (tt + 1) * 128],
                            wv_sb[:, c, :],
                            start=(c == 0), stop=(c == 7),
                        )
                    nc.tensor.matmul(
                        ga_ps[:], wqk_sb[:, 0, c, :], xT_sb[:, c, 0:512],
                        start=(c == 0), stop=(c == 7),
                    )
                for tt in range(2):
                    g1b_epilogue(gb_ps[tt], tt)
                nc.vector.tensor_scalar(
                    qkT[:, 0, 0:512], ga_ps[:], bqk_sb[:, 0:1],
                    None, mybir.AluOpType.add,
                )
                g1a_piece(4, 0)

                def flash_pair_qt(i, qt, pacer, reorder):
                    # heads 2i (psum partitions 0-63) and 2i+1 (64-127)
                    nkc = 4 * (qt + 1)
                    if reorder and qt > 0:
                        # diagonal (masked) chunks early: their longer
                        # exp->mask->PV chain gets extra pipeline slack,
                        # and the last PV of the qt is an unmasked chunk
                        kcs = ([0] + list(range(4 * qt, nkc))
                               + list(range(1, 4 * qt)))
                    else:
                        kcs = list(range(nkc))
                    a_ps = [
                        ps_a.tile([65, 512], f32, tag="aT", name=f"aps{i}_{qt}_{h}")
                        for h in range(2)
                    ]

                    def scores(kc):
                        # both heads' QK^T into one 2-bank psum tile, one
                        # fused exp, one fused causal mask. Diagonal chunks
                        # skip their fully-masked leading columns.
                        j = kc - 4 * qt
                        off = 128 * j if j > 0 else 0
                        sT = ps_s.tile(
                            [128, 2, 512], f32, tag="sT", name=f"sT{i}_{qt}_{kc}"
                        )
                        for h2 in range(2):
                            p0 = 64 * h2
                            qcols = slice(qt * 512 + off, (qt + 1) * 512)
                            kcols = slice(kc * 128, (kc + 1) * 128)
                            nc.tensor.matmul(
                                sT[:, h2, off:],
                                qkT[p0 : p0 + 64, 4 + i, kcols],
                                qkT[p0 : p0 + 64, i, qcols],
                                start=True, stop=True,
                            )
                        pT = ptmp.tile(
                            [128, 2, 512], bf16, tag="pT", name=f"pT{i}_{qt}_{kc}"
                        )
                        nc.scalar.activation(
                            pT[:, :, off:],
                            sT[:, :, off:],
                            mybir.ActivationFunctionType.Exp,
                            bias=zb[:],
                            scale=0.125,
                        )
                        if j >= 0:
                            nc.vector.tensor_tensor(
                                pT[:, :, off:],
                                pT[:, :, off:],
                                msk_sb[:, j : j + 1, off:].to_broadcast(
                                    (128, 2, 512 - off)
                                ),
                                mybir.AluOpType.mult,
                            )
                        return pT, off

                    def pv(kc, pT, off, first, last):
                        for h in range(2):
                            nc.tensor.matmul(
                                a_ps[h][:, off:],
                                von[:, kc, 2 * i + h, :],
                                pT[:, h, off:],
                                start=first,
                                stop=last,
                            )

                    # software pipeline: PV lags scores by two chunks; PE
                    # fillers (GEMM1a / c_proj pieces) absorb the exp/mask
                    # latency, stride-paced so they last the whole pair.
                    pend = []
                    npv = 0
                    for kc in kcs:
                        pend.append((kc, *scores(kc)))
                        if len(pend) > 2:
                            kc0, pT0, off0 = pend.pop(0)
                            pv(kc0, pT0, off0, npv == 0, False)
                            npv += 1
                            pacer.pump()
                    for (kc0, pT0, off0) in pend:
                        pv(kc0, pT0, off0, npv == 0, npv == nkc - 1)
                        npv += 1
                        pacer.pump()

                    # evacuate psum to SBUF first (frees the a_ps buffer
                    # for the next qt's PV), then normalize by the
                    # ones-row denominator off the critical path.
                    # Exception: the kernel's very last qt normalizes
                    # straight from psum — nothing competes for a_ps
                    # afterwards and the copy would lengthen the terminal
                    # normalize->AllGather->c_proj chain.
                    last_qt = i == 3 and qt == GQ - 1
                    for h in range(2):
                        if last_qt:
                            src = a_ps[h]
                        else:
                            src = ptmp2.tile([65, 512], f32, tag="acp",
                                             name=f"acp{i}_{qt}_{h}")
                            nc.vector.tensor_copy(src[:], a_ps[h][:])
                        rec = ptmp2.tile([1, 512], f32, tag="rec")
                        nc.vector.reciprocal(rec[:], src[64:65, :])
                        if i == 3 and not last_qt:
                            # PE-matmul broadcast: keeps the pair-3 norm
                            # chain off the Pool queue, where the AllGather
                            # collectives execute. (src is an SBUF copy
                            # here, so rb may live in PSUM — the DVE can
                            # only read one PSUM operand. The last qt reads
                            # src straight from PSUM and its preceding
                            # collective is long finished, so Pool is safe.)
                            rb = ps_g.tile([64, 512], f32, tag="g",
                                           name=f"rbp{i}_{qt}_{h}")
                            nc.tensor.matmul(rb[:], ones64[:], rec[:],
                                             start=True, stop=True)
                        else:
                            rb = ptmp2.tile([64, 512], f32, tag="rbs")
                            nc.gpsimd.partition_broadcast(rb[:], rec[:])
                        nc.vector.tensor_tensor(
                            aT[64 * h : 64 * h + 64, i, qt * 512 : (qt + 1) * 512],
                            src[0:64, :],
                            rb[:],
                            mybir.AluOpType.mult,
                        )

                def allgather_pair(i):
                    # ship pair i's attention output while later pairs compute
                    nc.sync.dma_start(ag_ins[i][:], aT[:, i, :])
                    if local_cc:
                        nc.sync.dma_start(ag_outs[i][0:128, :], ag_ins[i][:])
                        nc.sync.dma_start(ag_outs[i][128:256, :], ag_ins[i][:])
                    else:
                        nc.gpsimd.collective_compute(
                            "AllGather",
                            mybir.AluOpType.bypass,
                            replica_groups=groups,
                            ins=[ag_ins[i][:].opt()],
                            outs=[ag_outs[i][:].opt()],
                        )
                    for g in range(2):
                        nc.sync.dma_start(
                            gath[:, g * 4 + i, :],
                            ag_outs[i][g * 128 : (g + 1) * 128, :],
                        )

                def cc_ag(in_t, out_t):
                    if local_cc:
                        n = in_t.shape[0]
                        nc.sync.dma_start(out_t[0:n, :], in_t[:])
                        nc.sync.dma_start(out_t[n : 2 * n, :], in_t[:])
                    else:
                        nc.gpsimd.collective_compute(
                            "AllGather",
                            mybir.AluOpType.bypass,
                            replica_groups=groups,
                            ins=[in_t[:].opt()],
                            outs=[out_t[:].opt()],
                        )

                def allgather_01():
                    # merged pairs 0+1 AllGather (one fixed CC cost)
                    nc.sync.dma_start(ag_in01[:], aT[:, 0:2, :])
                    cc_ag(ag_in01, ag_out01)
                    for g in range(2):
                        for i in range(2):
                            nc.sync.dma_start(
                                gath[:, g * 4 + i, :],
                                ag_out01[g * 128 : (g + 1) * 128,
                                         i * S : (i + 1) * S],
                            )

                def allgather_2():
                    nc.sync.dma_start(ag_in2[:], aT[:, 2, :])
                    cc_ag(ag_in2, ag_out2)
                    for g in range(2):
                        nc.sync.dma_start(
                            gath[:, g * 4 + 2, :],
                            ag_out2[g * 128 : (g + 1) * 128, :],
                        )

                def allgather_3h(hh):
                    # pair-3 halves: qt0+qt1 after qt1, qt2+qt3 after qt3
                    cols = slice(hh * 1024, (hh + 1) * 1024)
                    nc.sync.dma_start(ag_in3h[hh][:], aT[:, 3, cols])
                    cc_ag(ag_in3h[hh], ag_out3h[hh])
                    for g in range(2):
                        nc.sync.dma_start(
                            gath[:, g * 4 + 3, cols],
                            ag_out3h[hh][g * 128 : (g + 1) * 128, :],
                        )

                def allgather_qt(i, qt):
                    # ship pair i's qt slice as soon as it's normalized
                    cols = slice(qt * 512, (qt + 1) * 512)
                    nc.sync.dma_start(ag_ins_q[i][qt][:], aT[:, i, cols])
                    if local_cc:
                        nc.sync.dma_start(
                            ag_outs_q[i][qt][0:128, :], ag_ins_q[i][qt][:])
                        nc.sync.dma_start(
                            ag_outs_q[i][qt][128:256, :], ag_ins_q[i][qt][:])
                    else:
                        nc.gpsimd.collective_compute(
                            "AllGather",
                            mybir.AluOpType.bypass,
                            replica_groups=groups,
                            ins=[ag_ins_q[i][qt][:].opt()],
                            outs=[ag_outs_q[i][qt][:].opt()],
                        )
                    for g in range(2):
                        nc.sync.dma_start(
                            gath[:, g * 4 + i, cols],
                            ag_outs_q[i][qt][g * 128 : (g + 1) * 128, :],
                        )

                def pass1_tt(tt):
                    # first half of c_proj contraction (pairs 0,1 of both
                    # groups); bf16 partial + bias into SBUF og16.
                    ps = ps_g.tile([128, 512], f32, tag="g")
                    for ci, c in enumerate([0, 4, 1, 5]):
                        nc.tensor.matmul(
                            ps[:],
                            gath[:, c, tt * 128 : (tt + 1) * 128],
                            wpj_sb[:, c, :],
                            start=(ci == 0), stop=(ci == 3),
                        )
                    nc.vector.tensor_tensor(
                        og16[:, tt, :], ps[:], bpj_sb[:], mybir.AluOpType.add
                    )

                def pass2a_tt(tt):
                    # contraction chunks {2,6}: needs only pair-2's AG, so
                    # it fills pair-3 qt0/qt1; accumulates into og16
                    ps = ps_g.tile([128, 512], f32, tag="g")
                    for ci, c in enumerate([2, 6]):
                        nc.tensor.matmul(
                            ps[:],
                            gath[:, c, tt * 128 : (tt + 1) * 128],
                            wpj_sb[:, c, :],
                            start=(ci == 0), stop=(ci == 1),
                        )
                    nc.vector.tensor_tensor(
                        og16[:, tt, :], ps[:], og16[:, tt, :],
                        mybir.AluOpType.add,
                    )

                def pass2_tt(tt, chunks=(3, 7), with_a=False, tail=False):
                    ps = ps_g.tile([128, 512], f32, tag="g")
                    cs = ((2, 6) + chunks) if with_a else chunks
                    for ci, c in enumerate(cs):
                        nc.tensor.matmul(
                            ps[:],
                            gath[:, c, tt * 128 : (tt + 1) * 128],
                            wpj_sb[:, c, :],
                            start=(ci == 0), stop=(ci == len(cs) - 1),
                        )
                    og = oev.tile([128, 512], f32, tag="og")
                    nc.vector.tensor_tensor(
                        og[:], ps[:], og16[:, tt, :], mybir.AluOpType.add
                    )
                    if tail and tail2:
                        # terminal chain: stores on the otherwise-idle
                        # scalar DMA queue (exp is finished; on hardware
                        # the two queues' transfers overlap)
                        nc.scalar.dma_start(
                            out[tt * 128 : (tt + 1) * 128, :], og[:])
                    else:
                        nc.sync.dma_start(
                            out[tt * 128 : (tt + 1) * 128, :], og[:])

                class Pacer:
                    """Spread filler parts evenly over the pump slots."""

                    def __init__(self, parts, slots, delay=0):
                        self.parts = list(parts)
                        self.frac = len(self.parts) / max(1, slots - delay)
                        self.acc = -delay * self.frac

                    def pump(self):
                        self.acc += self.frac
                        while self.acc >= 1.0 and self.parts:
                            self.acc -= 1.0
                            self.parts.pop(0)()

                    def drain(self):
                        for p in self.parts:
                            p()
                        self.parts.clear()

                # ---- schedule
                # g1a filler pieces per pair, deadline-ordered for pair 0
                # ((fc,t) complete before pair-0 qt t starts; (1,0),(5,0)
                # lead — pair 1 needs them first)
                pair_g1a = {
                    # (0,1),(4,1) lead: qt1's scores need them by pump ~4.
                    # g1b(2),(3) (von tt2/tt3) land before qt0's pv(kc=2,3).
                    0: [(0, 1), (4, 1), (1, 0), (5, 0)]
                    + [(f, t) for t in (2, 3) for f in (0, 4)]
                    + [(f, t) for t in (1, 2, 3) for f in (1, 5)]
                    + [(f, 0) for f in (2, 6)],
                    1: [(f, t) for t in (0, 1, 2, 3) for f in (3, 7)]
                    + [(f, t) for t in (1, 2, 3) for f in (2, 6)],
                }

                def G(f, t):
                    return lambda: g1a_piece(f, t)

                pair0_parts = (
                    [lambda: g1b_chunk(2), lambda: g1b_chunk(3)]
                    + [G(f, t) for (f, t) in pair_g1a[0]]
                )

                def P1(t):
                    return lambda: pass1_tt(t)

                def P2a(t):
                    return lambda: pass2a_tt(t)

                def P2b(t):
                    return lambda: pass2_tt(t)

                if ag == "qt":
                    # per-qt AllGathers land ~a qt after the data is
                    # produced, so each consumer (keyed on a single qt
                    # slice of gath) can fill one pair earlier
                    pair_pacer = {
                        0: (pair0_parts, len(pair0_parts), 0),
                        1: ([G(f, t) for (f, t) in pair_g1a[1]]
                            + [P1(t) for t in range(0, 4)], 40, 0),
                        2: ([P1(t) for t in range(4, 16)]
                            + [P2a(t) for t in range(0, 4)], 40, 2),
                        3: ([P2a(t) for t in range(4, 12)]
                            + [P2b(t) for t in range(0, 4)]
                            + [P2a(t) for t in range(12, 16)]
                            + [P2b(t) for t in range(4, 12)], 40, 1),
                    }

                for i in range(4):
                    if ag == "qt":
                        parts, slots, delay = pair_pacer[i]
                        pacer = Pacer(parts, slots, delay=delay)
                    elif i == 0:
                        pacer = Pacer(pair0_parts, len(pair0_parts))
                    elif i == 1:
                        pacer = Pacer([G(f, t) for (f, t) in pair_g1a[1]], 40)
                    elif i == 2:
                        # delay covers the pair-1 AllGather chain (write +
                        # collective + gathers) before the first pass1 part
                        pacer = Pacer(
                            [P1(t) for t in range(12)], 40,
                            delay=6 if ag == "m4" else 7)
                    for qt in range(GQ):
                        if i == 0 and qt > 0:
                            for tt in range(4 * qt, 4 * qt + 4):
                                g1b_chunk(tt)
                        if i == 3 and ag == "ag3":
                            if qt == 0:
                                # pass1 tt12-15: needs only pair-0/1 AGs,
                                # so it can't stall on the just-issued CC2
                                pacer = Pacer([P1(t) for t in range(12, 16)], 4)
                            elif qt == 1:
                                # {2,6} (CC2-gated, ~a qt of slack now) and
                                # {3,7} tt0-3 (AG3(qt0)-gated)
                                pacer = Pacer(
                                    [P2a(t) for t in range(4)]
                                    + [P2b(t) for t in range(4)], 8, delay=1)
                            elif qt == 2:
                                pacer = Pacer(
                                    [lambda t=t: pass2_tt(t, with_a=True)
                                     for t in range(4, 8)], 12, delay=2)
                            else:
                                pacer = Pacer(
                                    [lambda t=t: pass2_tt(t, with_a=True)
                                     for t in range(8, 12)], 16, delay=2)
                        elif i == 3 and ag == "m4":
                            if qt == 0:
                                pacer = Pacer([P1(t) for t in range(12, 16)], 4)
                            elif qt == 1:
                                pacer = Pacer(
                                    [P2a(t) for t in range(4)], 8, delay=1)
                            elif qt == 2:
                                pacer = Pacer(
                                    [P2b(t) for t in range(4)]
                                    + [lambda t=t: pass2_tt(t, with_a=True)
                                       for t in range(4, 6)], 12, delay=2)
                            else:
                                pacer = Pacer(
                                    [lambda t=t: pass2_tt(t, with_a=True)
                                     for t in range(6, 12)], 16, delay=1)
                        # pair 0 is reorder-safe too: (0,t)/(4,t) g1a pieces
                        # complete by pump t+2 (program order), ahead of
                        # qt t's early diagonal scores
                        flash_pair_qt(i, qt, pacer, reorder=(r0 or i > 0))
                        if ag == "qt":
                            allgather_qt(i, qt)
                        elif ag == "ag3" and i == 3:
                            pacer.drain()
                            allgather_qt(3, qt)
                        elif ag == "m4" and i == 3 and qt in (1, 3):
                            pacer.drain()
                            allgather_3h(qt // 2)
                    pacer.drain()
                    if ag == "ag3" and i < 3:
                        allgather_pair(i)
                    elif ag == "m4" and i == 1:
                        allgather_01()
                    elif ag == "m4" and i == 2:
                        allgather_2()

                # tail c_proj chunks
                if ag == "qt":
                    for tt in range(12, 16):
                        pass2_tt(tt, tail=True)
                else:
                    for tt in range(12, 16):
                        pass2_tt(tt, with_a=True, tail=True)

    nc.compile()
    _BUILD_CACHE[key] = nc
    return nc


def make_in_maps(x, w_attn, b_attn, w_proj, b_proj):
    """Shard the full inputs into 8 per-core input maps."""
    x = np.asarray(x, dtype=np.float32)
    w_attn = np.asarray(w_attn, dtype=np.float32)
    b_attn = np.asarray(b_attn, dtype=np.float32)
    w_proj = np.asarray(w_proj, dtype=np.float32)
    b_proj = np.asarray(b_proj, dtype=np.float32)

    kp = np.arange(128)[:, None, None]
    jj = np.arange(4)[None, :, None]
    qf = np.arange(512)[None, None, :]
    mask = (kp + 128 * jj <= qf).astype(BF16)

    in_maps = []
    for c in range(N_CORES):
        b, g = c // 2, c % 2
        sl = slice(g * FQK, (g + 1) * FQK)
        wq = w_attn[:, 0 * NX :][:, sl]
        wk = w_attn[:, 1 * NX :][:, sl]
        wv_ = w_attn[:, 2 * NX :][:, sl]
        bq = b_attn[0 * NX :][sl]
        bk = b_attn[1 * NX :][sl]
        bv_ = b_attn[2 * NX :][sl]

        def r128(a):
            # [NX, F] -> [128, 8, F] (partition-major for contiguous DMA)
            return np.ascontiguousarray(
                a.reshape(8, 128, a.shape[1]).transpose(1, 0, 2))

        xt = x[b].T
        # wqk fc-major: [part p, fc, c, j] <- w[c*128+p, fc*128+j]
        wqk_full = np.concatenate([wq, wk], axis=1)  # [1024, 1024]
        wqk_r = np.ascontiguousarray(
            wqk_full.reshape(8, 128, 8, 128).transpose(1, 2, 0, 3))
        in_maps.append(
            {
                "xT": r128(xt).astype(BF16),
                "wqk": wqk_r.astype(BF16),
                "wv": r128(wv_).astype(BF16),
                "bqk": np.ascontiguousarray(
                    np.concatenate([bq, bk]).reshape(8, 128).T
                ).astype(np.float32),
                "bvb": np.ascontiguousarray(
                    np.broadcast_to(bv_[None, :], (128, FQK))
                ).astype(np.float32),
                "wpj": r128(w_proj[:, sl]).astype(BF16),
                "bpj": np.ascontiguousarray(
                    np.broadcast_to(b_proj[None, sl], (128, FQK))
                ).astype(np.float32),
                "msk": mask,
            }
        )
    return in_maps


def assemble_out(results):
    out = np.empty((B, S, NX), dtype=np.float32)
    for c in range(N_CORES):
        b, g = c // 2, c % 2
        out[b, :, g * FQK : (g + 1) * FQK] = results[c]["out"]
    return out


def kernel(x, w_attn, b_attn, w_proj, b_proj):
    nc = build_nc()
    in_maps = make_in_maps(x, w_attn, b_attn, w_proj, b_proj)
    res = run_bass_kernel_spmd(nc, in_maps, core_ids=list(range(N_CORES)))
    return assemble_out(res.results)
